# revision 1
# baseline (speedup 1.0000x reference)
"""CSPN (convolutional spatial propagation network) kernel for Trainium2.

Reference computation (per batch image, 512x512, fp32):
  aff    = conv3x3(x, W_aff, SAME) + b_aff          # 8 channels
  a      = aff / sum_c |aff_c| ; s = sum_c a_c
  kernel = concat([1 - s, a])                       # 9 channels
  24 iterations:  x <- sum_k kernel_k * shift_{OFFS[k]}(x)   (zero padded)

Sharding: data-parallel over batch, one image per NeuronCore (8 cores).

Per-core design (everything SBUF resident, all fp32):
  * x state in two ping/pong buffers, layout [128 partitions, 6*514]:
    partition p holds image rows 4p..4p+3 in row slots 1..4 plus halo row
    slots 0 (row 4p-1) and 5 (row 4p+4); each row slot is 514 wide with a
    zero pad column each side.  Halo slots are refreshed after every
    iteration with two partition-shifted SBUF->SBUF DMAs, overlapped with
    the next iteration's centre-row products.
  * 9-channel diffusion kernel at [c][r][j] (free offset c*2048 + r*512 + j).
  * affinity conv on the vector engine: one fused scalar_tensor_tensor MAC
    per (channel, tap) — aff += W[c,a,b] * shifted x — with the 3x3 weights
    as per-partition scalars broadcast via a K=1 matmul through PSUM.
  * kernel generation: abs-reduce over channels, 2-ULP reciprocal, scale.
  * diffusion: 18 vector-engine tensor_tensor ops per iteration
    (9 products + 9 accumulate adds; final add split for halo overlap).
"""

import numpy as np

H = 512
W = 512
B = 8
ITER = 24
# itertools.product([0,1,-1], repeat=2) order (matches reference OFFS)
OFFS = [(i, j) for i in (0, 1, -1) for j in (0, 1, -1)]

WP = W + 2            # padded row width
NSLOT = 6             # row slots per partition (1 halo + 4 + 1 halo)
RJ = 4 * W            # 2048 free elems per channel plane per partition
AFF_CH = 8

_PROGRAM = None


def _build_program():
    import concourse.mybir as mybir
    from concourse import bacc, tile

    f32 = mybir.dt.float32
    mult = mybir.AluOpType.mult
    add = mybir.AluOpType.add

    nc = bacc.Bacc("TRN2", target_bir_lowering=False, debug=False, name="cspn")

    x_d = nc.dram_tensor("x", [H, W], f32, kind="ExternalInput")
    w_d = nc.dram_tensor("w_aff", [AFF_CH * 9], f32, kind="ExternalInput")
    b_d = nc.dram_tensor("b_aff", [AFF_CH], f32, kind="ExternalInput")
    out_d = nc.dram_tensor("out", [H, W], f32, kind="ExternalOutput")

    with tile.TileContext(nc) as tc:
        with (
            tc.tile_pool(name="state", bufs=1) as sp,
            tc.tile_pool(name="psum", bufs=1, space="PSUM") as pp,
        ):
            xb0 = sp.tile([128, NSLOT * WP], f32, tag="xb0")
            xb1 = sp.tile([128, NSLOT * WP], f32, tag="xb1")
            kern = sp.tile([128, 9 * RJ], f32, tag="kern")
            acc = sp.tile([128, RJ], f32, tag="acc")
            pr = sp.tile([128, RJ], f32, tag="pr")
            sums = sp.tile([128, RJ], f32, tag="sums")
            recip = sp.tile([128, RJ], f32, tag="recip")
            wbc = sp.tile([128, 80], f32, tag="wbc")
            ones = sp.tile([1, 128], f32, tag="ones")

            xv0 = xb0[:].rearrange("p (s w) -> p s w", w=WP)
            xv1 = xb1[:].rearrange("p (s w) -> p s w", w=WP)
            xviews = [xv0, xv1]

            # ---------------- init / loads ----------------
            nc.vector.memset(xb0[:], 0.0)
            nc.vector.memset(xb1[:], 0.0)
            nc.gpsimd.memset(ones[:], 1.0)

            nc.sync.dma_start(
                out=xv0[:, 1:5, 1 : 1 + W],
                in_=x_d.rearrange("(p r) w -> p r w", p=128),
            )
            # initial halo rows for xb0
            nc.sync.dma_start(out=xv0[1:128, 0, 1:513], in_=xv0[0:127, 4, 1:513])
            nc.sync.dma_start(out=xv0[0:127, 5, 1:513], in_=xv0[1:128, 1, 1:513])

            # w/b broadcast to all partitions via a K=1 matmul through PSUM
            nc.sync.dma_start(out=wbc[:1, :72], in_=w_d[None, :])
            nc.sync.dma_start(out=wbc[:1, 72:80], in_=b_d[None, :])
            pw = pp.tile([128, 80], f32, tag="wps")
            nc.tensor.matmul(pw[:, :], ones[:1, :], wbc[:1, :80],
                             start=True, stop=True)
            nc.vector.tensor_copy(out=wbc[:, :80], in_=pw[:, :])

            # ---------------- affinity conv (DVE, fused MAC per tap) -------
            # aff channel c lives in kern channel 1+c
            # aff_c[4p+r, j] = b_c + sum_{a,b} W[c,a,b] * x[4p+r+a-1, j+b-1]
            #   x row 4p+r+a-1 -> slot r+a ; col j+b-1 -> stored col j+b
            for c in range(AFF_CH):
                av = kern[:, (1 + c) * RJ : (2 + c) * RJ].rearrange(
                    "p (r j) -> p r j", j=W
                )
                for a in range(3):
                    for b3 in range(3):
                        xsh = xv0[:, a : a + 4, b3 : b3 + W]
                        wsc = wbc[:, c * 9 + a * 3 + b3 : c * 9 + a * 3 + b3 + 1]
                        if a == 0 and b3 == 0:
                            nc.vector.tensor_scalar(
                                out=av, in0=xsh, scalar1=wsc,
                                scalar2=wbc[:, 72 + c : 73 + c],
                                op0=mult, op1=add,
                            )
                        else:
                            nc.vector.scalar_tensor_tensor(
                                out=av, in0=xsh, scalar=wsc, in1=av,
                                op0=mult, op1=add,
                            )

            # ---------------- kernel generation ----------------
            aff = kern[:, RJ : 9 * RJ]
            affv = aff.rearrange("p (c rj) -> p rj c", c=AFF_CH)
            nc.vector.tensor_reduce(
                out=sums[:], in_=affv, axis=mybir.AxisListType.X,
                op=add, apply_absolute_value=True,
            )
            nc.vector.reciprocal_approx_accurate(recip[:], sums[:], scratch=acc[:])
            affc = aff.rearrange("p (c rj) -> p c rj", c=AFF_CH)
            rb = recip[:].unsqueeze(1).broadcast_to([128, AFF_CH, RJ])
            nc.vector.tensor_tensor(out=affc, in0=affc, in1=rb, op=mult)
            nc.vector.tensor_reduce(
                out=sums[:], in_=affv, axis=mybir.AxisListType.X, op=add,
            )
            # kern0 = 1 - s
            nc.vector.tensor_scalar(
                out=kern[:, 0:RJ], in0=sums[:], scalar1=-1.0, scalar2=1.0,
                op0=mult, op1=add,
            )

            # ---------------- diffusion ----------------
            accv = acc[:].rearrange("p (r j) -> p r j", j=W)
            prv = pr[:].rearrange("p (r j) -> p r j", j=W)
            for it in range(ITER):
                cur = xviews[it % 2]
                nxt = xviews[(it + 1) % 2]
                for c, (oi, oj) in enumerate(OFFS):
                    kv = kern[:, c * RJ : (c + 1) * RJ].rearrange(
                        "p (r j) -> p r j", j=W
                    )
                    xsh = cur[:, 1 - oi : 5 - oi, 1 - oj : 513 - oj]
                    if c == 0:
                        nc.vector.tensor_tensor(out=accv, in0=kv, in1=xsh, op=mult)
                    elif c < 8:
                        nc.vector.tensor_tensor(out=prv, in0=kv, in1=xsh, op=mult)
                        nc.vector.tensor_tensor(out=accv, in0=accv, in1=prv, op=add)
                    else:
                        nc.vector.tensor_tensor(out=prv, in0=kv, in1=xsh, op=mult)
                        # edge rows (r=0,3) first so halo DMAs launch early
                        nc.vector.tensor_tensor(
                            out=nxt[:, 1:5:3, 1:513], in0=accv[:, 0:4:3, :],
                            in1=prv[:, 0:4:3, :], op=add,
                        )
                        nc.sync.dma_start(
                            out=nxt[1:128, 0, 1:513], in_=nxt[0:127, 4, 1:513]
                        )
                        nc.sync.dma_start(
                            out=nxt[0:127, 5, 1:513], in_=nxt[1:128, 1, 1:513]
                        )
                        nc.vector.tensor_tensor(
                            out=nxt[:, 2:4, 1:513], in0=accv[:, 1:3, :],
                            in1=prv[:, 1:3, :], op=add,
                        )

            nc.sync.dma_start(
                out=out_d.rearrange("(p r) w -> p r w", p=128),
                in_=xviews[ITER % 2][:, 1:5, 1:513],
            )

    nc.finalize()
    return nc


def _get_program():
    global _PROGRAM
    if _PROGRAM is None:
        _PROGRAM = _build_program()
    return _PROGRAM


def kernel(x, W_aff, b_aff):
    from concourse.bass_utils import run_bass_kernel_spmd

    nc = _get_program()
    x = np.ascontiguousarray(np.asarray(x, dtype=np.float32))
    w = np.ascontiguousarray(np.asarray(W_aff, dtype=np.float32)).reshape(AFF_CH * 9)
    b = np.ascontiguousarray(np.asarray(b_aff, dtype=np.float32))

    in_maps = [{"x": x[i, 0], "w_aff": w, "b_aff": b} for i in range(B)]
    res = run_bass_kernel_spmd(nc, in_maps, list(range(B))).results
    out = np.stack([res[i]["out"] for i in range(B)], axis=0)[:, None]
    return out.astype(np.float32)



# revision 15
# speedup vs baseline: 4.1590x; 4.1590x over previous
"""CSPN (convolutional spatial propagation network) kernel for Trainium2.

Reference computation (per batch image, 512x512, fp32):
  aff    = conv3x3(x, W_aff, SAME) + b_aff          # 8 channels
  a      = aff / sum_c |aff_c| ; s = sum_c a_c
  kernel = concat([1 - s, a])                       # 9 channels
  24 iterations:  x <- sum_k kernel_k * shift_{OFFS[k]}(x)   (zero padded)

Sharding: data-parallel over batch, one image per NeuronCore (8 cores).

Per-core design (three-engine pipeline, f16 state):
  * x state in two ping/pong f16 buffers [128, 6*514]: partition p holds
    rows 4p..4p+3 in slots 1..4 plus halo slots 0/5, zero pad columns.
  * The output scale grows ~2.3x/iter (to ~7e8 > f16 max), so a 0.5x
    rescale per iteration is folded into the diffusion kernel (recip =
    0.5/abssum, center = 0.5 - 0.5*s); the final iteration's evacuation
    rescales by 2^24 while converting to fp32.
  * affinity conv on the PE: per (channel, tap) a W[c,tap]*I identity
    stationary (built on-device by DVE from eye x broadcast weights)
    matmul accumulating 9 taps into fp32 PSUM; Act evacuates aff (+bias)
    to f16 planes plus an Abs copy; DVE accumulates fp32 ksum/abssum
    under the conv.
  * kernel generation on DVE: recip = 0.5/abssum fp32, then f16-cast so
    the 8 normalize multiplies hit the 2x DVE mode; 9 f16 kernel planes,
    plane index t = 3b + a for tap (row a-1, col b-1).
  * diffusion iteration = 4 row-slot chunks in order (2,3,1,4):
      - 3 DVE f16 products per chunk (one per column tap b, 2x mode)
      - 9 PE identity matmuls accumulate the taps into fp32 PSUM
      - Act evacuates PSUM -> next-state slot (f16)
      - halo rows via PE shifted-identity matmuls + Act evacuation
        (measured ~9us/iter cheaper than SBUF->SBUF DMA halo exchange,
        and the shifted stationary zeroes the boundary partitions free)
    DVE/PE/Act run concurrently across chunks; the (2,3,1,4) order lets
    next-iteration products start before this iteration fully finishes.
"""

import numpy as np

H = 512
W = 512
B = 8
ITER = 24
# itertools.product([0,1,-1], repeat=2) order (matches reference OFFS)
OFFS = [(i, j) for i in (0, 1, -1) for j in (0, 1, -1)]
# diffusion kernel plane for reference channel c: t = 3*(1-oj) + (1-oi)
TMAP = [3 * (1 - oj) + (1 - oi) for (oi, oj) in OFFS]

WP = W + 2            # padded row width
RJ = 4 * W            # free elems per channel plane per partition
AFF_CH = 8

_PROGRAM = None


def _build_program(iters=ITER, hw_loop=0, conv_loop=0, no_conv=False):
    import concourse.mybir as mybir
    from concourse import bacc, tile

    f32 = mybir.dt.float32
    f16 = mybir.dt.float16
    mult = mybir.AluOpType.mult
    add = mybir.AluOpType.add
    Identity = mybir.ActivationFunctionType.Identity
    Abs = mybir.ActivationFunctionType.Abs
    Copy = mybir.ActivationFunctionType.Copy

    nc = bacc.Bacc("TRN2", target_bir_lowering=False, debug=False, name="cspn")

    x_d = nc.dram_tensor("x", [H, W], f32, kind="ExternalInput")
    # eyes: [0]=I, [1]=SH_UP (PSUM[po]=mov[po+1]), [2]=SH_DN (PSUM[po]=mov[po-1])
    eye_d = nc.dram_tensor("eye", [3, 128, 128], f16, kind="ExternalInput")
    wbc_d = nc.dram_tensor("wbc", [128, AFF_CH * 9], f32, kind="ExternalInput")
    bbc_d = nc.dram_tensor("bbc", [128, AFF_CH], f32, kind="ExternalInput")
    out_d = nc.dram_tensor("out", [H, W], f32, kind="ExternalOutput")

    with tile.TileContext(nc) as tc:
        with tc.tile_pool(name="state", bufs=1) as sp:
            xb0 = sp.tile([128, 6 * WP], f16, tag="xb0")
            xb1 = sp.tile([128, 6 * WP], f16, tag="xb1")
            kern = sp.tile([128, 9 * RJ], f16, tag="kern")
            eyes = sp.tile([128, 3 * 128], f16, tag="eyes")
            bbc = sp.tile([128, AFF_CH], f32, tag="bbc")
            xstage = sp.tile([128, RJ], f32, tag="xstage")
            ksum = sp.tile([128, RJ], f32, tag="ksum")
            absum = sp.tile([128, RJ], f32, tag="absum")
            recip = sp.tile([128, RJ], f32, tag="recip")
            scr = sp.tile([128, RJ], f32, tag="scr")

            xv0 = xb0[:].rearrange("p (s w) -> p s w", w=WP)
            xv1 = xb1[:].rearrange("p (s w) -> p s w", w=WP)
            xviews = [xv0, xv1]
            kv = kern[:].rearrange("p (t r w) -> p t r w", t=9, w=W)
            eyev = eyes[:].rearrange("p (k m) -> p k m", m=128)

            # ---------------- init / loads ----------------
            nc.vector.memset(xb0[:], 0.0)
            nc.vector.memset(xb1[:], 0.0)
            nc.sync.dma_start(
                out=eyes[:].rearrange("p (k m) -> p k m", m=128),
                in_=eye_d.rearrange("k q m -> q k m"),
            )
            nc.sync.dma_start(out=bbc[:], in_=bbc_d[:, :])
            nc.sync.dma_start(
                out=xstage[:], in_=x_d.rearrange("(p r) w -> p (r w)", p=128)
            )
            nc.vector.tensor_copy(
                out=xv0[:, 1:5, 1 : 1 + W],
                in_=xstage[:].rearrange("p (r w) -> p r w", w=W),
            )

            # initial halo rows for xb0 via PE partition shifts
            with tc.tile_pool(name="ihp", bufs=1, space="PSUM") as ihp:
                ph0 = ihp.tile([128, W], f32, tag="ph", name="ph0")
                ph1 = ihp.tile([128, W], f32, tag="ph2", name="ph1")
                nc.tensor.matmul(
                    ph0[:, :], eyev[:, 1, :], xv0[:, 1, 1 : 1 + W],
                    start=True, stop=True,
                )
                nc.scalar.activation(out=xv0[:, 5, 1 : 1 + W], in_=ph0[:, :], func=Copy)
                nc.tensor.matmul(
                    ph1[:, :], eyev[:, 2, :], xv0[:, 4, 1 : 1 + W],
                    start=True, stop=True,
                )
                nc.scalar.activation(out=xv0[:, 0, 1 : 1 + W], in_=ph1[:, :], func=Copy)

            if no_conv:
                nc.vector.memset(kern[:], 0.25)

            from contextlib import ExitStack as _ES

            _conv_scope = _ES()
            if not no_conv:
                cvp = _conv_scope.enter_context(
                    tc.tile_pool(name="cvp", bufs=1, space="PSUM")
                )
                cvs = _conv_scope.enter_context(tc.tile_pool(name="cvs", bufs=1))
            with _conv_scope:
              if not no_conv:
                # conv stationaries W[c,tap]*I built on-device (DVE, f16 2x)
                wbc = cvs.tile([128, AFF_CH * 9], f32, tag="wbc")
                stat = cvs.tile([128, AFF_CH * 9 * 128], f16, tag="stat")
                aff = cvs.tile([128, AFF_CH * RJ], f16, tag="aff")
                ab0 = cvs.tile([128, RJ], f16, tag="ab0")
                ab1 = cvs.tile([128, RJ], f16, tag="ab1")
                abt = [ab0, ab1]
                nc.sync.dma_start(out=wbc[:], in_=wbc_d[:, :])
                statv = stat[:].rearrange("p (k m) -> p k m", m=128)
                affv = aff[:].rearrange("p (c rj) -> p c rj", c=AFF_CH)

                def gen_stats(c):
                    for t in range(9):
                        k = c * 9 + t
                        nc.vector.tensor_scalar(
                            out=statv[:, k, :], in0=eyev[:, 0, :],
                            scalar1=wbc[:, k : k + 1], scalar2=0.0,
                            op0=mult, op1=add,
                        )

                from contextlib import ExitStack

                cv_ctx = ExitStack()
                if conv_loop:
                    cv_ctx.enter_context(tc.For_i(0, conv_loop))
                gen_stats(0)
                gen_stats(1)
                for c in range(AFF_CH):
                    if c + 2 < AFF_CH:
                        gen_stats(c + 2)
                    ps = cvp.tile(
                        [128, RJ], f32, tag=f"cv{c % 2}", name=f"cv{c % 2}"
                    )
                    psv = ps[:].rearrange("p (r w) -> p r w", w=W)
                    for r in range(4):
                        for t in range(9):
                            a, b3 = t % 3, t // 3
                            nc.tensor.matmul(
                                psv[:, r, :],
                                statv[:, c * 9 + t, :],
                                xv0[:, a + r, b3 : b3 + W],
                                start=(t == 0),
                                stop=(t == 8),
                            )
                    nc.scalar.activation(
                        out=affv[:, c, :], in_=ps[:, :], func=Identity,
                        bias=bbc[:, c : c + 1], scale=1.0,
                    )
                    nc.scalar.activation(
                        out=abt[c % 2][:], in_=ps[:, :], func=Abs,
                        bias=bbc[:, c : c + 1], scale=1.0,
                    )
                    # fp32 accumulation on DVE, overlapped under PE conv
                    if c == 0:
                        nc.vector.tensor_copy(out=ksum[:], in_=affv[:, 0, :])
                        nc.vector.tensor_copy(out=absum[:], in_=ab0[:])
                    else:
                        nc.vector.tensor_tensor(
                            out=ksum[:], in0=ksum[:], in1=affv[:, c, :], op=add
                        )
                        nc.vector.tensor_tensor(
                            out=absum[:], in0=absum[:], in1=abt[c % 2][:], op=add
                        )

                # ---------------- kernel generation (DVE) -------------
                # recip = 0.5/abssum  (via 1/(2*abssum)), then f16 for 2x mults
                nc.vector.tensor_scalar(
                    out=scr[:], in0=absum[:], scalar1=2.0, scalar2=0.0,
                    op0=mult, op1=add,
                )
                nc.vector.reciprocal_approx_accurate(
                    recip[:], scr[:], scratch=absum[:]
                )
                reciph = ab0
                nc.vector.tensor_copy(out=reciph[:], in_=recip[:])
                for c in range(AFF_CH):
                    nc.vector.tensor_tensor(
                        out=kv[:, TMAP[c + 1], :, :].rearrange("p r w -> p (r w)"),
                        in0=affv[:, c, :], in1=reciph[:], op=mult,
                    )
                nc.vector.tensor_tensor(
                    out=scr[:], in0=ksum[:], in1=recip[:], op=mult
                )
                nc.vector.tensor_scalar(
                    out=kv[:, TMAP[0], :, :].rearrange("p r w -> p (r w)"),
                    in0=scr[:], scalar1=-1.0, scalar2=0.5, op0=mult, op1=add,
                )
                cv_ctx.close()

            # ---------------- diffusion ----------------
            with (
                tc.tile_pool(name="dfp", bufs=1, space="PSUM") as dfp,
                tc.tile_pool(name="dfs", bufs=1) as dfs,
            ):
                pra = dfs.tile([128, 9 * W], f16, tag="pra")
                prb = dfs.tile([128, 9 * W], f16, tag="prb")
                prt = [pra, prb]
                pst = [
                    dfp.tile([128, W], f32, tag=f"df{s}", name=f"df{s}")
                    for s in range(4)
                ]
                hst = [
                    dfp.tile([128, W], f32, tag=f"hl{s}", name=f"hl{s}")
                    for s in range(2)
                ]

                def diffusion_iter(it, last=False):
                    cur = xviews[it % 2]
                    nxt = xviews[(it + 1) % 2]
                    for ci, s in enumerate((2, 3, 1, 4)):
                        pr = prt[ci % 2]
                        prv = pr[:].rearrange("p (t w) -> p t w", w=W)
                        for b3 in range(3):
                            nc.vector.tensor_tensor(
                                out=prv[:, 3 * b3 : 3 * b3 + 3, :],
                                in0=kv[:, 3 * b3 : 3 * b3 + 3, s - 1, :],
                                in1=cur[:, s - 1 : s + 2, b3 : b3 + W],
                                op=mult,
                            )
                        ps = pst[ci]
                        for t in range(9):
                            nc.tensor.matmul(
                                ps[:, :], eyev[:, 0, :], prv[:, t, :],
                                start=(t == 0), stop=(t == 8),
                            )
                        if last:
                            # fused output: f32 evac with 2^ITER rescale
                            nc.scalar.activation(
                                out=xstage[:, (s - 1) * W : s * W], in_=ps[:, :],
                                func=Copy, scale=float(2.0 ** ITER),
                            )
                            nc.sync.dma_start(
                                out=out_d.rearrange("(p r) w -> p r w", p=128)[
                                    :, s - 1, :
                                ],
                                in_=xstage[:, (s - 1) * W : s * W],
                            )
                            continue
                        nc.scalar.activation(
                            out=nxt[:, s, 1 : 1 + W], in_=ps[:, :], func=Copy
                        )
                        # halo rows via PE partition shift + Act evac
                        if s == 1:
                            nc.tensor.matmul(
                                hst[0][:, :], eyev[:, 1, :], nxt[:, 1, 1 : 1 + W],
                                start=True, stop=True,
                            )
                            nc.scalar.activation(
                                out=nxt[:, 5, 1 : 1 + W], in_=hst[0][:, :], func=Copy
                            )
                        elif s == 4:
                            nc.tensor.matmul(
                                hst[1][:, :], eyev[:, 2, :], nxt[:, 4, 1 : 1 + W],
                                start=True, stop=True,
                            )
                            nc.scalar.activation(
                                out=nxt[:, 0, 1 : 1 + W], in_=hst[1][:, :], func=Copy
                            )

                if hw_loop:
                    with tc.For_i(0, hw_loop):
                        for it in range(8):
                            diffusion_iter(it)
                    diffusion_iter(0, last=True)
                elif iters == 0:
                    nc.sync.dma_start(
                        out=out_d.rearrange("(p r) w -> p (r w)", p=128),
                        in_=xstage[:],
                    )
                else:
                    for it in range(iters):
                        diffusion_iter(it, last=(it == iters - 1))

    nc.finalize()
    return nc


def _get_program():
    global _PROGRAM
    if _PROGRAM is None:
        _PROGRAM = _build_program()
    return _PROGRAM


def _host_inputs(W_aff, b_aff):
    w = np.asarray(W_aff, dtype=np.float32).reshape(AFF_CH, 9)
    eye = np.stack(
        [
            np.eye(128, dtype=np.float16),
            np.eye(128, k=-1, dtype=np.float16),  # PSUM[po] = mov[po+1]
            np.eye(128, k=1, dtype=np.float16),   # PSUM[po] = mov[po-1]
        ]
    )
    # wbc[k = c*9 + t] = W[c, a*3 + b] for t = a + 3*b (tap row a-1, col b-1)
    wbc = np.empty((AFF_CH * 9,), np.float32)
    for c in range(AFF_CH):
        for t in range(9):
            a, b3 = t % 3, t // 3
            wbc[c * 9 + t] = w[c, a * 3 + b3]
    wbc = np.tile(wbc[None, :], (128, 1)).astype(np.float32)
    bbc = np.tile(
        np.asarray(b_aff, dtype=np.float32)[None, :], (128, 1)
    ).astype(np.float32)
    return eye, wbc, bbc


def kernel(x, W_aff, b_aff):
    from concourse.bass_utils import run_bass_kernel_spmd

    nc = _get_program()
    x = np.ascontiguousarray(np.asarray(x, dtype=np.float32))
    eye, wbc, bbc = _host_inputs(W_aff, b_aff)

    in_maps = [
        {"x": x[i, 0], "eye": eye, "wbc": wbc, "bbc": bbc} for i in range(B)
    ]
    res = run_bass_kernel_spmd(nc, in_maps, list(range(B))).results
    out = np.stack([res[i]["out"] for i in range(B)], axis=0)[:, None]
    return out.astype(np.float32)
